# revision 17
# baseline (speedup 1.0000x reference)
# Trainium2 Bass kernel for masked causal attention
#   B=2, H=16, S=2048, D=64, bool attn_mask [B, S, S] + causal, softmax, @V.
#
# Sharding: 8 cores x 4 heads (cores 0-3 -> batch 0, cores 4-7 -> batch 1).
#
# Each head's causal score region (17408 q-columns across 16 k-tiles) is
# processed in a PACKED layout split into two q-halves, as 12 psum slots of
# [128, 1536]: QK matmuls pack score columns densely, exp runs as wide
# back-to-back ACT instructions into a packed fp16 p-buffer, the bool mask
# (pre-packed on host) is applied with wide DVE multiplies, and PV matmuls
# accumulate [V | ones] @ p into per-bank [65, 512] psum tiles (row 64 =
# softmax denominator). Banks stream out as soon as they stop accumulating;
# the denominator division happens on the HOST.
#
# Engine balance (PE ~58us of QK+PV streaming is the intended pacer; every
# other engine is kept below it so the PE never idles and the HAM clock
# gate stays at 2.4 GHz):
#  - The tail 512 columns of slots 1,3,5,7,9 skip the ACT exp: their QK
#    matmuls add a 65th contraction row (q-row=1, k-row=82.875) so the
#    psum holds st + B/A, and one DVE scalar_tensor_tensor computes
#    int16_bits(p) = round((st + B/A)*A) * mask  -- a Schraudolph fp16
#    exp approximation (rel err ~1.8% rms on ~15% of columns) that also
#    applies the mask for free. ACT ~57us.
#  - DVE mask windows skip the Schraudolph ranges. DVE ~55us.
#  - GPSIMD multiplies the final [15360,17408) window of each head (its
#    PVs are tail-scheduled anyway; keeps DVE under the PE).

import numpy as np

B, H, S, D = 2, 16, 2048, 64
NCORES = 8
HPC = 4
P = 128
NKT = S // P
SLOT = 1536
PACKED = sum(S - P * j for j in range(NKT))   # 17408
NSLOT = 12
SLOT_W = [1536] * 11 + [512]
SLOT_OFF = [1536 * i for i in range(12)]
PV_BUDGET = 1792

SCHRAUD_SLOTS = (1, 3, 5, 7, 9)               # slots whose [1024,1536) tail
SCH_LO = 1024                                 # is computed via Schraudolph
SCH_BA = 82.875                               # fp16-exact bias row value
SCH_A = float(0.125 * 1024 * np.log2(np.e))   # bits = (st + BA) * A

_cache = {}

# --- static piece tables (identical for every head) -----------------------
SEGS = []
_off = 0
for _v in range(2):
    for _j in range(NKT):
        _qs, _qe = max(P * _j, 1024 * _v), 1024 * (_v + 1)
        if _qs < _qe:
            SEGS.append((_v, _j, _off, _qs, _qe - _qs))
            _off += _qe - _qs
assert _off == PACKED

# QK pieces per slot; 'bias' marks Schraudolph pieces (65-row operands)
QK_SLOTS = [[] for _ in range(NSLOT)]   # slot -> [(j, dst_off, q0, w, bias)]
for (_v, _j, _o, _qs, _w) in SEGS:
    _a = _o
    while _a < _o + _w:
        _b = min(_o + _w, (_a // 512 + 1) * 512)
        _s = _a // SLOT
        _doff = _a - _s * SLOT
        _bias = _s in SCHRAUD_SLOTS and _doff >= SCH_LO
        QK_SLOTS[_s].append((_j, _doff, _qs + (_a - _o), _b - _a, _bias))
        _a = _b

# DVE mask windows (skip the Schraudolph ranges, which the stt masks) and
# the GPSIMD final window. (a, b, close_slot, gp)
MWIN = [(0, 2560, 1, False), (3072, 5632, 3, False), (6144, 8704, 5, False),
        (9216, 11776, 7, False), (12288, 14848, 9, False),
        (15360, 16896, 10, True), (16896, 17408, 11, True)]
MW_BY_CLOSE = {}
for _i, _mw in enumerate(MWIN):
    MW_BY_CLOSE.setdefault(_mw[2], []).append(_mw)

# PV readiness regions: packed range end -> close slot of the op that masks
# its last column. elig = close + 3 (DVE) ; last two regions go to tails.
_REG = [(3072, 1), (6144, 3), (9216, 5), (12288, 7), (15360, 9), (16896, 10),
        (17408, 11)]

PV_ELIG = {}
PV_TAIL = [[], []]
for (_v, _j, _o, _qs, _w) in SEGS:
    for _b in (2 * _v, 2 * _v + 1):
        _q0, _q1 = max(P * _j, 512 * _b), 512 * (_b + 1)
        if _q0 >= _q1:
            continue
        _pa = _o + _q0 - _qs
        _pb = _pa + _q1 - _q0
        _close = next(c for e, c in _REG if e >= _pb)
        _r = _close + 3
        _pc = (_j, _b, _q0, _q1, _pa, _pb, _j == 0,
               _j == min(4 * _b + 3, NKT - 1))
        if _r <= NSLOT - 1:
            PV_ELIG.setdefault(_r, []).append(_pc)
        else:
            PV_TAIL[0 if _close <= 9 else 1].append(_pc)
for _r in PV_ELIG:
    PV_ELIG[_r].sort(key=lambda p: (p[1] // 2, p[0], p[1]))
for _t in PV_TAIL:
    _t.sort(key=lambda p: (p[1] // 2, p[0], p[1]))


def build_nc():
    import concourse.bacc as bacc
    import concourse.mybir as mybir
    import concourse.tile as tile
    from concourse import library_config
    from contextlib import ExitStack

    fp16 = mybir.dt.float16
    i16 = mybir.dt.int16
    f32 = mybir.dt.float32
    Exp = mybir.ActivationFunctionType.Exp
    Mult = mybir.AluOpType.mult

    nc = bacc.Bacc("TRN2", target_bir_lowering=False, debug=False,
                   num_devices=NCORES)

    qt_d = nc.dram_tensor("qt", [HPC, D + 1, S], fp16, kind="ExternalInput")
    kt_d = nc.dram_tensor("kt", [HPC, D + 1, S], fp16, kind="ExternalInput")
    vp_d = nc.dram_tensor("vp", [HPC, P, NKT, D + 1], fp16, kind="ExternalInput")
    mk_d = nc.dram_tensor("maskp", [P, PACKED], fp16, kind="ExternalInput")
    out_d = nc.dram_tensor("outt", [HPC, D + 1, S], f32, kind="ExternalOutput")

    with tile.TileContext(nc) as tc, ExitStack() as ctx:
        mask_pool = ctx.enter_context(tc.tile_pool(name="mask", bufs=1))
        qk_pool = ctx.enter_context(tc.tile_pool(name="qk", bufs=2))
        vp_pool = ctx.enter_context(tc.tile_pool(name="vpool", bufs=2))
        p_pool = ctx.enter_context(tc.tile_pool(name="p", bufs=2))
        o_pool = ctx.enter_context(tc.tile_pool(name="osb", bufs=2))
        c_pool = ctx.enter_context(tc.tile_pool(name="cst", bufs=1))
        st_psum = ctx.enter_context(tc.tile_pool(name="st", bufs=2, space="PSUM"))
        o_psum = ctx.enter_context(tc.tile_pool(name="outp", bufs=2, space="PSUM"))

        nc.gpsimd.load_library(library_config.standard)

        # PE warm-up: ~4us of matmuls on zeros so the HAM clock gate opens
        # to 2.4 GHz just as the real QK stream begins.
        wsb = c_pool.tile([P, 512], fp16, tag="warm")
        nc.vector.memset(wsb[:], 0.0)
        wps = st_psum.tile([P, SLOT], f32, tag="st")
        for i in range(9):
            lo = 512 * (i % 3)
            nc.tensor.matmul(wps[:, lo:lo + 512], lhsT=wsb[:, 0:128],
                             rhs=wsb[:], start=True, stop=True)

        def load_head(h):
            qt = qk_pool.tile([D + 1, S], fp16, tag="qt")
            nc.sync.dma_start(qt[:], qt_d[h])
            kt = qk_pool.tile([D + 1, S], fp16, tag="kt")
            nc.sync.dma_start(kt[:], kt_d[h])
            vp = vp_pool.tile([P, NKT, D + 1], fp16, tag="vp")
            nc.sync.dma_start(vp[:], vp_d[h])
            return qt, kt, vp

        head_tiles = {0: load_head(0)}
        mask_sb = mask_pool.tile([P, PACKED], fp16, tag="mask")
        for g in range(4):
            nc.sync.dma_start(mask_sb[:, SLOT_OFF[g]:SLOT_OFF[g] + SLOT_W[g]],
                              mk_d[:, SLOT_OFF[g]:SLOT_OFF[g] + SLOT_W[g]])

        prev_tail = None

        for h in range(HPC):
            qt, kt, vp = head_tiles.pop(h, None) or load_head(h)
            obank = [o_psum.tile([D + 1, 512], f32, tag="ob", name=f"ob{h}_{b}")
                     for b in range(4)]
            p = p_pool.tile([P, PACKED], fp16, tag="p")
            osb = o_pool.tile([D + 1, S], f32, tag="osb")
            backlog = []

            def emit_pv(piece, vp=vp, p=p, obank=obank, osb=osb, h=h):
                j, b, q0, q1, pa, pb, st_, sp_ = piece
                nc.tensor.matmul(obank[b][:, q0 - 512 * b:q1 - 512 * b],
                                 lhsT=vp[:, j, :], rhs=p[:, pa:pb],
                                 start=st_, stop=sp_)
                if sp_:
                    nc.vector.tensor_copy(osb[:, 512 * b:512 * (b + 1)],
                                          obank[b][:])
                    nc.sync.dma_start(out_d[h, :, 512 * b:512 * (b + 1)],
                                      osb[:, 512 * b:512 * (b + 1)])

            # ramp fillers: keep the PE dense through the pipeline-fill ramp
            # so the HAM clock gate opens early (garbage into obank; the
            # first real PV's start=True clears it). Slots 2+ emit them
            # BEFORE the QK pieces, which wait on the st-buffer recycle.
            # (warm slots must stay < 4: the first real PV accumulations
            # begin at slot 4 and a later start=True would clear them)
            WARM_BEFORE = {2: 4, 3: 4} if h == 0 else {}
            WARM_AFTER = {0: 2, 1: 2} if h == 0 else {}

            def emit_warm(n, s, obank=obank):
                for wb in range(n):
                    nc.tensor.matmul(obank[(s + wb) % 4][:, 0:512],
                                     lhsT=wsb[:, 0:D + 1], rhs=wsb[:],
                                     start=True, stop=True)

            for s in range(NSLOT):
                st = st_psum.tile([P, SLOT], f32, tag="st")
                emit_warm(WARM_BEFORE.get(s, 0), s)
                for (j, off, q0, w, bias) in QK_SLOTS[s]:
                    rows = D + 1 if bias else D
                    nc.tensor.matmul(st[:, off:off + w],
                                     lhsT=kt[0:rows, j * P:(j + 1) * P],
                                     rhs=qt[0:rows, q0:q0 + w],
                                     start=True, stop=True)
                emit_warm(WARM_AFTER.get(s, 0), s)
                so = SLOT_OFF[s]
                sw = SLOT_W[s]
                if s in SCHRAUD_SLOTS:
                    nc.scalar.activation(p[:, so:so + SCH_LO], st[:, :SCH_LO],
                                         Exp, scale=0.125)
                    nc.vector.scalar_tensor_tensor(
                        p[:, so + SCH_LO:so + sw].bitcast(i16),
                        st[:, SCH_LO:sw], SCH_A,
                        mask_sb[:, so + SCH_LO:so + sw], Mult, Mult)
                else:
                    nc.scalar.activation(p[:, so:so + sw], st[:, :sw],
                                         Exp, scale=0.125)
                for (a, b_, cl, gp) in MW_BY_CLOSE.get(s, []):
                    # last head: no next head hides the slow GPSIMD window,
                    # so run it on the (by then idle) DVE instead
                    eng = nc.gpsimd if gp and h + 1 < HPC else nc.vector
                    eng.tensor_mul(p[:, a:b_], p[:, a:b_], mask_sb[:, a:b_])
                if h == 0 and s + 4 < NSLOT:
                    g = s + 4
                    nc.sync.dma_start(
                        mask_sb[:, SLOT_OFF[g]:SLOT_OFF[g] + SLOT_W[g]],
                        mk_d[:, SLOT_OFF[g]:SLOT_OFF[g] + SLOT_W[g]])
                if prev_tail is not None and s in (3, 5):
                    k = (s - 3) // 2
                    prev_tail[k]()
                    if k == len(prev_tail) - 1:
                        prev_tail = None
                backlog.extend(PV_ELIG.get(s, []))
                budget = PV_BUDGET if s < NSLOT - 1 else 10 ** 9
                while backlog and budget > 0:
                    piece = backlog.pop(0)
                    budget -= piece[5] - piece[4]
                    emit_pv(piece)
                if s == 4 and h + 1 < HPC:
                    head_tiles[h + 1] = load_head(h + 1)

            def mk_tail(emit_pv=emit_pv):
                t0 = lambda: [emit_pv(pc) for pc in PV_TAIL[0]]
                t1 = lambda: [emit_pv(pc) for pc in PV_TAIL[1]]
                return [t0, t1]

            prev_tail = mk_tail()

        if prev_tail is not None:
            for t in prev_tail:
                t()

    nc.compile()
    return nc


def prep_inputs(query, key, value, attn_mask):
    """Host-side layout prep (transposes/retiling/casts only) -> 8 in_maps."""
    query = np.asarray(query, dtype=np.float32)
    key = np.asarray(key, dtype=np.float32)
    value = np.asarray(value, dtype=np.float32)
    attn_mask = np.asarray(attn_mask).astype(bool)

    qT = np.ascontiguousarray(query.transpose(0, 1, 3, 2)).astype(np.float16)
    kT = np.ascontiguousarray(key.transpose(0, 1, 3, 2)).astype(np.float16)
    # row 64: Schraudolph bias (q-side ones, k-side B/A)
    qTb = np.concatenate(
        [qT, np.ones((B, H, 1, S), np.float16)], axis=2)
    kTb = np.concatenate(
        [kT, np.full((B, H, 1, S), SCH_BA, np.float16)], axis=2)

    vp = np.concatenate(
        [value, np.ones((B, H, S, 1), np.float32)], axis=3).astype(np.float16)
    vp = np.ascontiguousarray(
        vp.reshape(B, H, NKT, P, D + 1).transpose(0, 1, 3, 2, 4))

    tril = np.tril(np.ones((S, S), dtype=bool))
    in_maps = []
    for b in range(B):
        mT = (attn_mask[b] & tril).T.astype(np.float16)   # [k, q]
        maskp = np.empty((P, PACKED), np.float16)
        for (v, j, o, qs, w) in SEGS:
            maskp[:, o:o + w] = mT[P * j:P * (j + 1), qs:qs + w]
        for cl in range(NCORES // B):
            h0 = cl * HPC
            in_maps.append({
                "qt": np.ascontiguousarray(qTb[b, h0:h0 + HPC]),
                "kt": np.ascontiguousarray(kTb[b, h0:h0 + HPC]),
                "vp": np.ascontiguousarray(vp[b, h0:h0 + HPC]),
                "maskp": maskp,
            })
    return in_maps


def run(query, key, value, attn_mask, trace=False, trace_cores=None):
    from concourse import bass_utils

    if "nc" not in _cache:
        _cache["nc"] = build_nc()
    nc = _cache["nc"]

    in_maps = prep_inputs(query, key, value, attn_mask)
    res = bass_utils.run_bass_kernel_spmd(
        nc, in_maps, core_ids=list(range(NCORES)),
        trace=trace, trace_cores=trace_cores)

    out = np.empty((B, H, S, D), np.float32)
    for c in range(NCORES):
        b = c // (NCORES // B)
        h0 = (c % (NCORES // B)) * HPC
        outt = res.results[c]["outt"]          # [HPC, 65, S]
        num = outt[:, 0:D, :]
        den = outt[:, D:D + 1, :]
        out[b, h0:h0 + HPC] = (num / den).transpose(0, 2, 1)
    return out, res


def kernel(query, key, value, attn_mask):
    out, _ = run(query, key, value, attn_mask)
    return out
